# revision 17
# baseline (speedup 1.0000x reference)
"""Trainium2 Bass kernel for nn_Attention_19035295056566.

Dense transformer block: qkv = x @ w_qkv; head-axis attention (6x6 per
token); torch-faithful dim-mixing transpose; proj = out_mix @ w_proj + b.

Distribution: pure data-parallel over batch B=8 across the 8 NeuronCores
(one batch element per core, weights replicated, no collectives).

Self-contained: hardcodes shapes B=8, N=4096, C=768, H=6, D=128.
"""

import sys

for _p in ("/opt/trn_rl_repo",):
    if _p not in sys.path:
        sys.path.insert(0, _p)

import numpy as np
import ml_dtypes

from concourse import bass, bacc, mybir, tile

F32 = mybir.dt.float32
BF16 = mybir.dt.bfloat16

B, N_TOK, C = 8, 4096, 768
H, D = 6, 128
SCALE = float(D) ** -0.5
NCH = C // 128  # 6 c-chunks

TILE = 128  # tokens per tile (softmax/AV granularity)
GRP = 16  # tokens per QK pack group (16*6 = 96 rows)
NG = TILE // GRP  # groups per tile = 8


def build_graph(n_tok=N_TOK, chunk=512, debug=False, taps=False, reps=1, level=5,
                bufs=None, av_wide=False, bias_zero=False, dma_split=False,
                diag_gps=True, loop_reps=None):
    """Build the single-core Bass graph (same graph runs SPMD on 8 cores).

    reps>1 repeats the whole computation (for differential timing).
    """
    nc = bacc.Bacc("TRN2", target_bir_lowering=False, debug=debug)

    x_d = nc.dram_tensor("x", [n_tok, C], F32, kind="ExternalInput")
    wqkv_d = nc.dram_tensor("w_qkv", [C, 3 * C], F32, kind="ExternalInput")
    wproj_d = nc.dram_tensor("w_proj", [C, C], F32, kind="ExternalInput")
    bproj_d = nc.dram_tensor("b_proj", [C], F32, kind="ExternalInput")
    ident_d = nc.dram_tensor("ident", [128, 128], BF16, kind="ExternalInput")
    out_d = nc.dram_tensor("out", [n_tok, C], F32, kind="ExternalOutput")

    n_chunks = n_tok // chunk
    nr = chunk // TILE  # token tiles per chunk
    n_tiles = n_tok // TILE

    tap_d = {}
    if taps:
        tap_d["q_il"] = nc.dram_tensor(
            "tap_q_il", [n_chunks, 128, chunk * H], BF16, kind="ExternalOutput"
        )
        tap_d["k_il"] = nc.dram_tensor(
            "tap_k_il", [n_chunks, 128, chunk * H], BF16, kind="ExternalOutput"
        )
        tap_d["v_nat"] = nc.dram_tensor(
            "tap_v_nat", [n_chunks, 128, nr * C], BF16, kind="ExternalOutput"
        )
        tap_d["attn"] = nc.dram_tensor(
            "tap_attn", [n_tiles, 128, H * H], F32, kind="ExternalOutput"
        )
        tap_d["p"] = nc.dram_tensor(
            "tap_p", [n_tiles, 128, H * H], F32, kind="ExternalOutput"
        )
        tap_d["outT"] = nc.dram_tensor(
            "tap_outT", [128, H * n_tok], BF16, kind="ExternalOutput"
        )

    bu = dict(sb=2, small=3, psum=2, psumT=2, psumA=2, dram=3)
    if bufs:
        bu.update(bufs)
    with tile.TileContext(nc) as tc:
        with (
            tc.tile_pool(name="const", bufs=1) as constp,
            tc.tile_pool(name="sb", bufs=bu["sb"]) as sbp,
            tc.tile_pool(name="small", bufs=bu["small"]) as smallp,
            tc.tile_pool(name="outt", bufs=1) as outtp,
            tc.tile_pool(name="psum", bufs=bu["psum"], space="PSUM") as psp,
            tc.tile_pool(name="psumT", bufs=bu["psumT"], space="PSUM") as pstp,
            tc.tile_pool(name="psumA", bufs=bu["psumA"], space="PSUM") as psap,
            tc.tile_pool(name="dram", bufs=bu["dram"], space="DRAM") as dramp,
        ):
            # ---- constants ----
            wqkv_sb = constp.tile([128, NCH, 3 * C], BF16)
            for ch in range(NCH):
                nc.gpsimd.dma_start(
                    out=wqkv_sb[:, ch, :], in_=wqkv_d[128 * ch : 128 * (ch + 1), :]
                )
            wproj_sb = constp.tile([128, NCH, C], BF16)
            for j in range(NCH):
                nc.gpsimd.dma_start(
                    out=wproj_sb[:, j, :], in_=wproj_d[128 * j : 128 * (j + 1), :]
                )
            ident = constp.tile([128, 128], BF16)
            nc.sync.dma_start(out=ident[:], in_=ident_d[:])
            bias_row = constp.tile([1, C], F32)
            nc.sync.dma_start(out=bias_row[:], in_=bproj_d.ap().unsqueeze(0))
            bias_sb = constp.tile([128, C], F32)
            nc.gpsimd.partition_broadcast(bias_sb[:], bias_row[:])

            # attention output, transposed: column m = h*n_tok + t holds
            # out_attn[t, h, :] over the 128 d-partitions. bf16.
            outT = outtp.tile([128, H * n_tok], BF16)
            outT_ht = outT[:].rearrange("p (h t) -> p h t", h=H)  # write view

            def copy_op(i, out, in_):
                if i % 2 == 0:
                    nc.vector.tensor_copy(out, in_)
                else:
                    nc.scalar.copy(out, in_)

            def _all_reps():
                for _rep in range(reps):
                    _run_body(
                        nc, tc, copy_op, n_tok, chunk, taps, tap_d,
                        sbp, smallp, psp, pstp, psap, dramp,
                        wqkv_sb, wproj_sb, ident, bias_sb, outT, outT_ht,
                        x_d, out_d, level, av_wide, bias_zero, dma_split,
                        diag_gps,
                    )

            if loop_reps:
                with tc.For_i(0, loop_reps):
                    _all_reps()
            else:
                _all_reps()

    nc.compile()
    return nc


def _run_body(
    nc, tc, copy_op, n_tok, chunk, taps, tap_d,
    sbp, smallp, psp, pstp, psap, dramp,
    wqkv_sb, wproj_sb, ident, bias_sb, outT, outT_ht,
    x_d, out_d, level=5, av_wide=False, bias_zero=False, dma_split=False,
    diag_gps=True,
):
    n_chunks = n_tok // chunk
    nr = chunk // TILE
    n_tiles = n_tok // TILE
    if True:
        if True:
            for cc in range(n_chunks):
                t0 = cc * chunk

                # ---- load x (cast f32->bf16 during DMA) ----
                x_bf = sbp.tile([128, nr, C], BF16, tag="x_bf")
                for rg in range(nr):
                    nc.gpsimd.dma_start(
                        out=x_bf[:, rg, :],
                        in_=x_d[t0 + TILE * rg : t0 + TILE * (rg + 1), :],
                    )

                # ---- transpose x -> xT [c, tokens] ----
                xT = sbp.tile([128, NCH, chunk], BF16, tag="xT")
                for rg in range(nr):
                    for ch in range(NCH):
                        pt = pstp.tile([128, 128], BF16, tag="tp")
                        nc.tensor.transpose(
                            pt[:], x_bf[:, rg, 128 * ch : 128 * (ch + 1)], ident[:]
                        )
                        copy_op(
                            rg * NCH + ch,
                            xT[:, ch, TILE * rg : TILE * (rg + 1)],
                            pt[:],
                        )

                # ---- q, k matmuls (transposed out), interleaved store ----
                # q_il/k_il: [128 d, chunk, H] bf16; flat col = t*H + h
                q_il = sbp.tile([128, chunk, H], BF16, tag="q_il")
                k_il = sbp.tile([128, chunk, H], BF16, tag="k_il")
                for qk in range(2):
                    dst = q_il if qk == 0 else k_il
                    for h in range(H):
                        col0 = (qk * H + h) * 128
                        ps = psp.tile([128, chunk], F32, tag="big")
                        for ch in range(NCH):
                            nc.tensor.matmul(
                                ps[:],
                                wqkv_sb[:, ch, col0 : col0 + 128],
                                xT[:, ch, :],
                                start=(ch == 0),
                                stop=(ch == NCH - 1),
                            )
                        copy_op(qk * H + h, dst[:, :, h], ps[:])

                # ---- v matmul (natural layout) ----
                v_nat = sbp.tile([128, nr, C], BF16, tag="v_nat")
                for rg in range(nr):
                    for half in range(2):
                        psv = psp.tile([128, 384], F32, tag="big")
                        c0 = 2 * C + 384 * half
                        for ch in range(NCH):
                            nc.tensor.matmul(
                                psv[:],
                                xT[:, ch, TILE * rg : TILE * (rg + 1)],
                                wqkv_sb[:, ch, c0 : c0 + 384],
                                start=(ch == 0),
                                stop=(ch == NCH - 1),
                            )
                        copy_op(
                            rg * 2 + half,
                            v_nat[:, rg, 384 * half : 384 * (half + 1)],
                            psv[:],
                        )

                if level == 1:
                    nc.gpsimd.dma_start(out=out_d[0:1, :], in_=v_nat[0:1, 0, :])
                    nc.gpsimd.dma_start(
                        out=out_d[1:2, :], in_=q_il[0:1, 0:128, :]
                    )
                    nc.gpsimd.dma_start(
                        out=out_d[2:3, :], in_=k_il[0:1, 0:128, :]
                    )
                    continue

                q_flat = q_il[:].rearrange("p t h -> p (t h)")
                k_flat = k_il[:].rearrange("p t h -> p (t h)")

                if taps:
                    nc.sync.dma_start(out=tap_d["q_il"][cc], in_=q_flat)
                    nc.sync.dma_start(out=tap_d["k_il"][cc], in_=k_flat)
                    nc.sync.dma_start(
                        out=tap_d["v_nat"][cc],
                        in_=v_nat[:].rearrange("p r c -> p (r c)"),
                    )

                if level < 2:
                    continue

                # ---- Phase 1: QK^T pack-16 + extraction for all tiles ----
                # (keeps the PE stream free of the per-tile latency chain:
                # psum copy -> DRAM bounce -> gather -> softmax -> diag)
                attns = []
                for rg in range(nr):
                    qk_stage = smallp.tile([96, NG, 96], F32, tag="qk_stage")
                    for gi in range(NG):
                        e0 = (rg * TILE + gi * GRP) * H
                        pq = (pstp if av_wide else psap).tile(
                            [96, 96], F32, tag="tp" if av_wide else "qkatt"
                        )
                        nc.tensor.matmul(
                            pq[:],
                            q_flat[:, e0 : e0 + 96],
                            k_flat[:, e0 : e0 + 96],
                            start=True,
                            stop=True,
                        )
                        copy_op(gi, qk_stage[:, gi, :], pq[:])

                    qkd = dramp.tile([96, NG, 96], F32, tag="qkd")
                    (nc.scalar if dma_split else nc.sync).dma_start(
                        out=qkd[:], in_=qk_stage[:]
                    )

                    # attn_raw[tok, h, g] = qkd[6t+h, gi, 6t+g], tok=16*gi+t
                    attn_raw = smallp.tile([128, H, H], F32, tag="attn_raw")
                    qkd_ap = qkd[:]
                    for h in range(H):
                        srcap = bass.AP(
                            tensor=qkd_ap.tensor,
                            offset=qkd_ap.offset + h * (NG * 96),
                            ap=[[96, NG], [6 * NG * 96 + 6, GRP], [1, H]],
                        )
                        geng = (
                            (nc.scalar if h % 2 else nc.sync)
                            if dma_split
                            else nc.sync
                        )
                        geng.dma_start(out=attn_raw[:, h, :], in_=srcap)
                    attns.append(attn_raw)

                # ---- Phase 2: softmax + diag build (DVE/ACT/GpSimd only) ----
                diags = []
                for rg in range(nr):
                    tt = cc * nr + rg
                    attn_raw = attns[rg]
                    ex = smallp.tile([128, H, H], F32, tag="ex")
                    nc.scalar.activation(
                        ex[:],
                        attn_raw[:],
                        mybir.ActivationFunctionType.Exp,
                        scale=SCALE,
                    )
                    zsum = smallp.tile([128, H], F32, tag="zsum")
                    nc.vector.tensor_reduce(
                        zsum[:],
                        ex[:],
                        axis=mybir.AxisListType.X,
                        op=mybir.AluOpType.add,
                    )
                    rcp = smallp.tile([128, H], F32, tag="rcp")
                    nc.vector.reciprocal(rcp[:], zsum[:])
                    p_t = smallp.tile([128, H, H], F32, tag="p_t")
                    nc.vector.tensor_tensor(
                        p_t[:],
                        ex[:],
                        rcp[:].unsqueeze(-1).broadcast_to([128, H, H]),
                        op=mybir.AluOpType.mult,
                    )
                    p_flat = p_t[:].rearrange("p h g -> p (h g)")

                    if taps:
                        nc.sync.dma_start(
                            out=tap_d["attn"][tt],
                            in_=attn_raw[:].rearrange("p h g -> p (h g)"),
                        )
                        nc.sync.dma_start(out=tap_d["p"][tt], in_=p_flat)

                    if level < 3:
                        nc.sync.dma_start(
                            out=out_d[0:1, 0 : H * H], in_=p_flat[0:1, :]
                        )
                        continue
                    diag = smallp.tile([128, H * H, 128], BF16, tag="diag")
                    for hg in range(H * H):
                        if diag_gps:
                            which = hg % 3
                        else:
                            # DVE-heavy 2-way split (GpSimd per-op overhead
                            # suspected): DVE 2 of 3, ACT 1 of 3
                            which = 1 if hg % 3 == 1 else 0
                        if which == 0:
                            nc.vector.tensor_scalar(
                                diag[:, hg, :],
                                ident[:],
                                p_flat[:, hg : hg + 1],
                                None,
                                op0=mybir.AluOpType.mult,
                            )
                        elif which == 1:
                            nc.scalar.activation(
                                diag[:, hg, :],
                                ident[:],
                                mybir.ActivationFunctionType.Copy,
                                scale=p_flat[:, hg : hg + 1],
                            )
                        else:
                            nc.gpsimd.tensor_scalar(
                                diag[:, hg, :],
                                ident[:],
                                p_flat[:, hg : hg + 1],
                                None,
                                op0=mybir.AluOpType.mult,
                            )
                    diags.append(diag)

                if level < 3:
                    continue

                # ---- Phase 3: AV matmuls (PE) ----
                for rg in range(nr):
                    tt = cc * nr + rg
                    diag = diags[rg]
                    if level < 4:
                        nc.gpsimd.dma_start(
                            out=out_d[0:1, 0:128], in_=diag[0:1, 0, :]
                        )
                        continue
                    if av_wide:
                        diag_v = diag[:].rearrange("p (h g) t -> p h g t", h=H)
                        av_a = psap.tile([128, 512], F32, tag="av")
                        av_b = psap.tile([128, 256], F32, tag="av")
                        for g in range(H):
                            vg = v_nat[:, rg, 128 * g : 128 * (g + 1)]
                            nc.tensor.matmul(
                                av_a[:],
                                vg,
                                diag_v[:, 0:4, g, :],
                                start=(g == 0),
                                stop=(g == H - 1),
                            )
                            nc.tensor.matmul(
                                av_b[:],
                                vg,
                                diag_v[:, 4:6, g, :],
                                start=(g == 0),
                                stop=(g == H - 1),
                            )
                        nc.vector.tensor_copy(
                            outT_ht[:, 0:4, TILE * tt : TILE * (tt + 1)],
                            av_a[:].rearrange("p (h t) -> p h t", h=4),
                        )
                        nc.scalar.copy(
                            outT_ht[:, 4:6, TILE * tt : TILE * (tt + 1)],
                            av_b[:].rearrange("p (h t) -> p h t", h=2),
                        )
                    else:
                        for h in range(H):
                            av = psap.tile([128, 128], F32, tag="av")
                            for g in range(H):
                                nc.tensor.matmul(
                                    av[:],
                                    v_nat[:, rg, 128 * g : 128 * (g + 1)],
                                    diag[:, h * H + g, :],
                                    start=(g == 0),
                                    stop=(g == H - 1),
                                )
                            copy_op(
                                h,
                                outT_ht[:, h, TILE * tt : TILE * (tt + 1)],
                                av[:],
                            )

            if taps:
                nc.sync.dma_start(out=tap_d["outT"][:], in_=outT[:])

            # ---- proj: y[n', o] = sum_j OM[6n'+j] @ Wj + b ----
            if level < 5:
                # sink to defeat DCE: write something cheap to out_d
                if level >= 4:
                    nc.gpsimd.dma_start(
                        out=out_d[0:128, 0:128], in_=outT_ht[:, 0, 0:128]
                    )
                else:
                    nc.sync.dma_start(out=out_d[0:128, :], in_=bias_sb[:])
                return
            omT = outT[:].rearrange("p (i six) -> p i six", six=H)
            for t in range(n_tiles):
                ya = psp.tile([128, 384], F32, tag="big")
                yb = psp.tile([128, 384], F32, tag="big")
                for j in range(H):
                    lhsT = omT[:, TILE * t : TILE * (t + 1), j]
                    nc.tensor.matmul(
                        ya[:],
                        lhsT,
                        wproj_sb[:, j, 0:384],
                        start=(j == 0),
                        stop=(j == H - 1),
                    )
                    nc.tensor.matmul(
                        yb[:],
                        lhsT,
                        wproj_sb[:, j, 384:768],
                        start=(j == 0),
                        stop=(j == H - 1),
                    )
                y_sb = smallp.tile([128, C], F32, tag="y_sb")
                if bias_zero:
                    nc.vector.tensor_copy(y_sb[:, 0:384], ya[:])
                    nc.scalar.copy(y_sb[:, 384:768], yb[:])
                else:
                    nc.vector.scalar_tensor_tensor(
                        y_sb[:, 0:384],
                        ya[:],
                        1.0,
                        bias_sb[:, 0:384],
                        op0=mybir.AluOpType.mult,
                        op1=mybir.AluOpType.add,
                    )
                    nc.vector.scalar_tensor_tensor(
                        y_sb[:, 384:768],
                        yb[:],
                        1.0,
                        bias_sb[:, 384:768],
                        op0=mybir.AluOpType.mult,
                        op1=mybir.AluOpType.add,
                    )
                nc.sync.dma_start(
                    out=out_d[TILE * t : TILE * (t + 1), :], in_=y_sb[:]
                )


def build_graph2(n_tok=N_TOK, chunk=512, debug=False, reps=1, level=5,
                 bias_zero=False, loop_reps=None, stt_gps_mod=6,
                 copy_split=2, qk_stage_bufs=2):
    """v2: software-pipelined chunk loop.

    Pipeline stages per chunk c (emitted with offsets so PE never waits
    on same-chunk vector work):
      A-load(c):  x DMA
      A(c):       xT transposes, q/k/v matmuls, QK matmuls + bounce +
                  gather + softmax
      B(c):       AV via per-partition scalar_tensor_tensor chains (no
                  diag matmuls), outT transposes
      P(c):       proj tiles whose scattered token deps complete at c
    """
    nc = bacc.Bacc("TRN2", target_bir_lowering=False, debug=debug)

    x_d = nc.dram_tensor("x", [n_tok, C], F32, kind="ExternalInput")
    wqkv_d = nc.dram_tensor("w_qkv", [C, 3 * C], F32, kind="ExternalInput")
    wproj_d = nc.dram_tensor("w_proj", [C, C], F32, kind="ExternalInput")
    bproj_d = nc.dram_tensor("b_proj", [C], F32, kind="ExternalInput")
    ident_d = nc.dram_tensor("ident", [128, 128], BF16, kind="ExternalInput")
    out_d = nc.dram_tensor("out", [n_tok, C], F32, kind="ExternalOutput")

    n_chunks = n_tok // chunk
    nr = chunk // TILE
    n_tiles = n_tok // TILE

    # proj tile -> last chunk its inputs depend on (dim-mixing transpose
    # scatters token deps; see maxchunk analysis)
    tiles_by_chunk = {c: [] for c in range(n_chunks)}
    for tp in range(n_tiles):
        mc = 0
        for nn in (128 * tp, 128 * (tp + 1) - 1):
            for j in (0, 5):
                mc = max(mc, ((6 * nn + j) % n_tok) // chunk)
        tiles_by_chunk[mc].append(tp)

    with tile.TileContext(nc) as tc:
        with (
            tc.tile_pool(name="const", bufs=1) as constp,
            tc.tile_pool(name="sb", bufs=2) as sbp,
            tc.tile_pool(name="stg", bufs=qk_stage_bufs) as stgp,
            tc.tile_pool(name="small", bufs=2) as smallp,
            tc.tile_pool(name="outt", bufs=1) as outtp,
            tc.tile_pool(name="psum", bufs=2, space="PSUM") as psp,
            tc.tile_pool(name="psumT", bufs=2, space="PSUM") as pstp,
            tc.tile_pool(name="psumP", bufs=2, space="PSUM") as pprj,
            tc.tile_pool(name="psumA", bufs=2, space="PSUM") as psap,
            tc.tile_pool(name="dram", bufs=2, space="DRAM") as dramp,
        ):
            wqkv_sb = constp.tile([128, NCH, 3 * C], BF16)
            for ch in range(NCH):
                nc.gpsimd.dma_start(
                    out=wqkv_sb[:, ch, :], in_=wqkv_d[128 * ch : 128 * (ch + 1), :]
                )
            wproj_sb = constp.tile([128, NCH, C], BF16)
            for j in range(NCH):
                nc.gpsimd.dma_start(
                    out=wproj_sb[:, j, :], in_=wproj_d[128 * j : 128 * (j + 1), :]
                )
            ident = constp.tile([128, 128], BF16)
            nc.sync.dma_start(out=ident[:], in_=ident_d[:])
            bias_row = constp.tile([1, C], F32)
            nc.sync.dma_start(out=bias_row[:], in_=bproj_d.ap().unsqueeze(0))
            bias_sb = constp.tile([128, C], F32)
            nc.gpsimd.partition_broadcast(bias_sb[:], bias_row[:])

            outT = outtp.tile([128, H * n_tok], BF16)
            outT_ht = outT[:].rearrange("p (h t) -> p h t", h=H)

            def copy_op(i, out, in_):
                if i % copy_split == 0:
                    nc.vector.tensor_copy(out, in_)
                else:
                    nc.scalar.copy(out, in_)

            def _all_reps2():
                for _rep_i in range(reps):
                    _run_body2(
                        nc, tc, copy_op, n_tok, chunk, level,
                        sbp, stgp, smallp, psp, pstp, pprj, psap, dramp,
                        wqkv_sb, wproj_sb, ident, bias_sb, outT, outT_ht,
                        x_d, out_d, bias_zero, tiles_by_chunk, stt_gps_mod,
                    )

            if loop_reps:
                with tc.For_i(0, loop_reps):
                    _all_reps2()
            else:
                _all_reps2()

    nc.compile()
    return nc


def _run_body2(
    nc, tc, copy_op, n_tok, chunk, level,
    sbp, stgp, smallp, psp, pstp, pprj, psap, dramp,
    wqkv_sb, wproj_sb, ident, bias_sb, outT, outT_ht,
    x_d, out_d, bias_zero, tiles_by_chunk, stt_gps_mod,
):
    n_chunks = n_tok // chunk
    nr = chunk // TILE
    row_pitch = nr * NG * 96  # qkd row stride (f32 elems)

    st = {}  # per-chunk state: tiles live across pipeline stages

    def stage_load(c):
        x_bf = sbp.tile([128, nr, C], BF16, tag="x_bf")
        for rg in range(nr):
            nc.gpsimd.dma_start(
                out=x_bf[:, rg, :],
                in_=x_d[c * chunk + TILE * rg : c * chunk + TILE * (rg + 1), :],
            )
        st[c] = {"x_bf": x_bf}

    def stage_qkv(c):
        s = st[c]
        x_bf = s["x_bf"]
        xT = sbp.tile([128, NCH, chunk], BF16, tag="xT")
        for rg in range(nr):
            for ch in range(NCH):
                pt = pstp.tile([128, 128], BF16, tag="tp")
                nc.tensor.transpose(
                    pt[:], x_bf[:, rg, 128 * ch : 128 * (ch + 1)], ident[:]
                )
                copy_op(
                    rg * NCH + ch, xT[:, ch, TILE * rg : TILE * (rg + 1)], pt[:]
                )

        q_il = sbp.tile([128, chunk, H], BF16, tag="q_il")
        k_il = sbp.tile([128, chunk, H], BF16, tag="k_il")
        for qk in range(2):
            dst = q_il if qk == 0 else k_il
            for h in range(H):
                col0 = (qk * H + h) * 128
                ps = psp.tile([128, chunk], F32, tag="big")
                for ch in range(NCH):
                    nc.tensor.matmul(
                        ps[:],
                        wqkv_sb[:, ch, col0 : col0 + 128],
                        xT[:, ch, :],
                        start=(ch == 0),
                        stop=(ch == NCH - 1),
                    )
                copy_op(qk * H + h, dst[:, :, h], ps[:])

        v_nat = sbp.tile([128, nr, C], BF16, tag="v_nat")
        for rg in range(nr):
            for half in range(2):
                psv = psp.tile([128, 384], F32, tag="big")
                c0 = 2 * C + 384 * half
                for ch in range(NCH):
                    nc.tensor.matmul(
                        psv[:],
                        xT[:, ch, TILE * rg : TILE * (rg + 1)],
                        wqkv_sb[:, ch, c0 : c0 + 384],
                        start=(ch == 0),
                        stop=(ch == NCH - 1),
                    )
                copy_op(
                    rg * 2 + half,
                    v_nat[:, rg, 384 * half : 384 * (half + 1)],
                    psv[:],
                )
        s.update(q_il=q_il, k_il=k_il, v_nat=v_nat)

    def stage_qk_softmax(c):
        s = st[c]
        q_flat = s["q_il"][:].rearrange("p t h -> p (t h)")
        k_flat = s["k_il"][:].rearrange("p t h -> p (t h)")

        # qk_stage layout [96 rows, NG, 96 cols, nr]: rg innermost so the
        # gather's (g, rg) dims merge into one contiguous run (the DMA AP
        # balancer caps APs at 3 dims).
        qk_stage = stgp.tile([96, NG, 96, nr], F32, tag="qk_stage")
        for rg in range(nr):
            for gi in range(NG):
                e0 = (rg * TILE + gi * GRP) * H
                pq = psap.tile([96, 96], F32, tag="qkatt")
                nc.tensor.matmul(
                    pq[:],
                    q_flat[:, e0 : e0 + 96],
                    k_flat[:, e0 : e0 + 96],
                    start=True,
                    stop=True,
                )
                copy_op(rg * NG + gi, qk_stage[:, gi, :, rg], pq[:])

        qkd = dramp.tile([96, NG, 96, nr], F32, tag="qkd")
        nc.sync.dma_start(
            out=qkd[:],
            in_=qk_stage[:].rearrange("p g x r -> p (g x r)"),
        )

        # gather diagonal blocks: attn[t=16*gi+tt, h, g, rg] =
        #   qkd[6*tt+h, gi, 6*tt+g, rg]   ((g, rg) contiguous on both sides)
        attn_c = smallp.tile([128, H, H, nr], F32, tag="attn_c")
        qkd_ap = qkd[:]
        rp = NG * 96 * nr  # row pitch in f32 elems
        for h in range(H):
            srcap = bass.AP(
                tensor=qkd_ap.tensor,
                offset=qkd_ap.offset + h * rp,
                ap=[
                    [96 * nr, NG],
                    [6 * rp + 6 * nr, GRP],
                    [1, H * nr],
                ],
            )
            nc.sync.dma_start(
                out=attn_c[:, h, :, :].rearrange("p g r -> p (g r)"), in_=srcap
            )

        ex = smallp.tile([128, H, H, nr], F32, tag="ex")
        nc.scalar.activation(
            ex[:], attn_c[:], mybir.ActivationFunctionType.Exp, scale=SCALE
        )
        zsum = smallp.tile([128, H, nr], F32, tag="zsum")
        nc.vector.tensor_reduce(
            zsum[:],
            ex[:].rearrange("p h g r -> p h r g"),
            axis=mybir.AxisListType.X,
            op=mybir.AluOpType.add,
        )
        rcp = smallp.tile([128, H, nr], F32, tag="rcp")
        nc.vector.reciprocal(rcp[:], zsum[:])
        p_c = smallp.tile([128, H, H, nr], F32, tag="p_c")
        nc.vector.tensor_tensor(
            p_c[:],
            ex[:],
            rcp[:].unsqueeze(2).broadcast_to([128, H, H, nr]),
            op=mybir.AluOpType.mult,
        )
        s["p_c"] = p_c

    def stage_av(c):
        # Accumulation chains interleaved g-major: adjacent DVE queue
        # entries belong to different (rg, h) chains, so the engine
        # pipelines at throughput instead of stalling on each in-place
        # op's latency.
        s = st[c]
        v_nat, p_c = s["v_nat"], s["p_c"]
        oa = sbp.tile([128, nr, H, 128], BF16, tag="oa")
        for g in range(H):
            for rg in range(nr):
                for h in range(H):
                    dst = oa[:, rg, h, :]
                    vg = v_nat[:, rg, 128 * g : 128 * (g + 1)]
                    pg = p_c[:, h, g, rg : rg + 1]
                    if g == 0:
                        nc.vector.tensor_scalar(
                            dst, vg, pg, None, op0=mybir.AluOpType.mult
                        )
                    else:
                        nc.vector.scalar_tensor_tensor(
                            dst, vg, pg, dst,
                            op0=mybir.AluOpType.mult,
                            op1=mybir.AluOpType.add,
                        )
        s["oa"] = oa

    def stage_outT(c):
        s = st[c]
        oa = s["oa"]
        for rg in range(nr):
            tt = c * nr + rg
            for h in range(H):
                pt = pstp.tile([128, 128], BF16, tag="tp")
                nc.tensor.transpose(pt[:], oa[:, rg, h, :], ident[:])
                copy_op(
                    rg * H + h,
                    outT_ht[:, h, TILE * tt : TILE * (tt + 1)],
                    pt[:],
                )
        # chunk fully consumed: allow pools to rotate
        del st[c]["x_bf"]

    def stage_proj(c):
        omT = outT[:].rearrange("p (i six) -> p i six", six=H)
        for t in tiles_by_chunk[c]:
            ya = pprj.tile([128, 384], F32, tag="prj")
            yb = pprj.tile([128, 384], F32, tag="prj")
            for j in range(H):
                lhsT = omT[:, TILE * t : TILE * (t + 1), j]
                nc.tensor.matmul(
                    ya[:], lhsT, wproj_sb[:, j, 0:384],
                    start=(j == 0), stop=(j == H - 1),
                )
                nc.tensor.matmul(
                    yb[:], lhsT, wproj_sb[:, j, 384:768],
                    start=(j == 0), stop=(j == H - 1),
                )
            y_sb = smallp.tile([128, C], F32, tag="y_sb")
            if bias_zero:
                nc.vector.tensor_copy(y_sb[:, 0:384], ya[:])
                nc.scalar.copy(y_sb[:, 384:768], yb[:])
            else:
                nc.vector.scalar_tensor_tensor(
                    y_sb[:, 0:384], ya[:], 1.0, bias_sb[:, 0:384],
                    op0=mybir.AluOpType.mult, op1=mybir.AluOpType.add,
                )
                nc.vector.scalar_tensor_tensor(
                    y_sb[:, 384:768], yb[:], 1.0, bias_sb[:, 384:768],
                    op0=mybir.AluOpType.mult, op1=mybir.AluOpType.add,
                )
            nc.sync.dma_start(
                out=out_d[TILE * t : TILE * (t + 1), :], in_=y_sb[:]
            )

    # ---- pipelined emission ----
    for it in range(n_chunks + 2):
        cL, cA, cB = it, it - 1, it - 2
        if cB >= 0 and level >= 3:
            stage_av(cB)
        if cL < n_chunks:
            stage_load(cL)
        if 0 <= cA < n_chunks:
            stage_qkv(cA)
        if cB >= 0 and level >= 4:
            stage_outT(cB)
        if 0 <= cA < n_chunks and level >= 2:
            stage_qk_softmax(cA)
        if cB >= 0 and level >= 5:
            stage_proj(cB)

    # sinks to defeat DCE at diagnostic levels
    if level < 5:
        if level >= 4:
            nc.gpsimd.dma_start(out=out_d[0:128, 0:128], in_=outT_ht[:, 0, 0:128])
        elif level >= 3:
            last = st[n_chunks - 1]
            nc.gpsimd.dma_start(
                out=out_d[0:128, :], in_=last["oa"][:, 0, :, :].rearrange("p h d -> p (h d)")
            )
        elif level >= 2:
            for c in range(n_chunks):
                nc.gpsimd.dma_start(
                    out=out_d[c : c + 1, 0 : nr * H * H],
                    in_=st[c]["p_c"][0:1, :, :, :].rearrange("p h g r -> p (h g r)"),
                )
        else:
            for c in range(n_chunks):
                s = st[c]
                nc.gpsimd.dma_start(out=out_d[3 * c : 3 * c + 1, :], in_=s["v_nat"][0:1, 0, :])
                nc.gpsimd.dma_start(
                    out=out_d[3 * c + 1 : 3 * c + 2, :], in_=s["q_il"][0:1, 0:128, :]
                )
                nc.gpsimd.dma_start(
                    out=out_d[3 * c + 2 : 3 * c + 3, :], in_=s["k_il"][0:1, 0:128, :]
                )


_CACHED = {}

# flags chosen by on-hardware A/B (see bench.py)
BEST = dict(dma_split=False, diag_gps=False)


def _get_graph(n_tok=N_TOK, chunk=512, **kw):
    key = (n_tok, chunk, tuple(sorted(kw.items())))
    if key not in _CACHED:
        _CACHED[key] = build_graph(n_tok, chunk, **kw)
    return _CACHED[key]


def make_in_map(x_i, w_qkv, w_proj, b_proj):
    return {
        "x": np.ascontiguousarray(x_i, dtype=np.float32),
        "w_qkv": np.asarray(w_qkv, dtype=np.float32),
        "w_proj": np.asarray(w_proj, dtype=np.float32),
        "b_proj": np.asarray(b_proj, dtype=np.float32),
        "ident": np.eye(128, dtype=ml_dtypes.bfloat16),
    }


def kernel(x, w_qkv, w_proj, b_proj):
    from concourse.bass_utils import run_bass_kernel_spmd

    x = np.asarray(x, dtype=np.float32)
    nc = _get_graph(
        bias_zero=bool(np.all(np.asarray(b_proj) == 0.0)), **BEST
    )
    in_maps = [make_in_map(x[i], w_qkv, w_proj, b_proj) for i in range(B)]
    res = run_bass_kernel_spmd(nc, in_maps, core_ids=list(range(B)))
    out = np.stack([res.results[i]["out"] for i in range(B)], axis=0)
    return out.astype(np.float32)


if __name__ == "__main__":
    rng = np.random.default_rng(0)
    x = rng.standard_normal((B, N_TOK, C), dtype=np.float32)
    w_qkv = (rng.standard_normal((C, 3 * C)) * C**-0.5).astype(np.float32)
    w_proj = (rng.standard_normal((C, C)) * C**-0.5).astype(np.float32)
    b_proj = np.zeros((C,), dtype=np.float32)
    y = kernel(x=x, w_qkv=w_qkv, w_proj=w_proj, b_proj=b_proj)
    print(y.shape, y.dtype)

